# revision 23
# baseline (speedup 1.0000x reference)
"""Barycentric interpolation kernel for Trainium2 (8 NeuronCores, SPMD).

Problem: out[m] = (sum_k c[m,k]*fi[k]*wi[k]) / (sum_k c[m,k]*wi[k]),
c[m,k] = 1/(x[m]-xi[k]) with exact-hit override, N=64 nodes, M=2^21 queries.

Strategy (per core, data-parallel over m):
  * Nodes are grouped into 32 adjacent PAIRS. For pair p with roots (a,b):
      1/(x-a)/(x-b) = r2 = 1/q2,  q2 = (x-a)(x-b)
      fw_a/(x-a) + fw_b/(x-b) = (alpha_f*x + beta_f) * r2
    so numer = x*A + B, denom = x*C + D where
      A = sum_p alpha_f[p]*r2_p   (and B, C, D similarly)
    are computed by the TensorEngine from r2 rows only.
  * r2 is produced by 3 fused custom DVE ops (product + flip-seed + 3
    Newton-Raphson steps total, ~1-2 ulp).
  * Work layout: rows = 32 pairs x 4 query groups (K=128), queries along
    the free dim; x is broadcast into this layout by DMA.
  * PE matmul with a block-diagonal [128,16] stationary yields
    A,B,C,D per query group; ScalarE stages PSUM->SBUF; DVE epilogue does
    numer/denom with the stock approx-reciprocal (accurate variant).
  * Exact node hits (q2 == +-0) propagate NaN through the pipeline and are
    fixed up on the host with a faithful f32 formula.
"""

import os
import re
import sys
import types
import numpy as np

import concourse.bass as bass
import concourse.mybir as mybir
import concourse.tile as tile
from concourse.bass_utils import run_bass_kernel_spmd


def _ensure_ntff_hook():
    """The agent image's antenv lacks axon_hooks; synthesize it so
    run_bass_kernel_spmd(trace=True) can capture NTFF profiles."""
    try:
        from antenv.axon_hooks import get_axon_ntff_profile_hook  # noqa: F401
        return
    except ImportError:
        pass
    try:
        import antenv
        from trn_agent_boot.trn_boot import _ntff_profile_via_ctypes
        mod = types.ModuleType("antenv.axon_hooks")
        _state = {"hook": None}
        mod.set_axon_ntff_profile_hook = lambda h: _state.__setitem__("hook", h)
        mod.get_axon_ntff_profile_hook = lambda: _state["hook"]
        sys.modules["antenv.axon_hooks"] = mod
        antenv.axon_hooks = mod
        so_path = "/opt/axon/libaxon_pjrt.so"
        if os.path.exists(so_path):
            mod.set_axon_ntff_profile_hook(_ntff_profile_via_ctypes(so_path))
    except Exception:
        pass


_ensure_ntff_hook()


def _install_walrus_compat():
    """This image's walrus predates two concourse behaviors:
    1. Multi-wait drains ("Too many sync wait commands"): the Tile kernel-tail
       drain carries one sync-wait per tracked proc; this walrus's CTRL
       encoding takes at most one. Chunk the waits across drains and skip
       the EVENT_SEMAPHORE_RANGE_CLEAR-based sem recycling (also unsupported).
    2. InstISA subclasses (custom DVE ops) need .instr bytes populated
       client-side via codegen_inst_isa_subclasses ("ISA wrong length").
    """
    import bass_rust
    from concourse.vector_clock import ScopedClock

    if getattr(tile.TileContext, "_bary_compat", False):
        return

    def _compat_drain_and_barrier(self, tick_clock, wait_clock):
        drain_inst = self.nc.sync.drain()
        wait_clock.add_sem_waits(
            drain_inst.ins, ScopedClock({None: tick_clock.global_clock})
        )
        d = drain_inst.ins
        waits = list(d.sync_info.on_wait)
        if len(waits) > 1 and getattr(self, "_bary_walrus_compat", True):
            d.sync_info = bass_rust.SyncInfo(on_wait=waits[:1], on_update=[])
            for w in waits[1:]:
                d2 = self.nc.sync.drain()
                d2.ins.sync_info = bass_rust.SyncInfo(on_wait=[w], on_update=[])
        self.nc.all_engine_barrier()
        popped = self.nc._tile_sem_poison_stack.pop()
        assert popped is self._sem_poison
        self.nc.all_engine_barrier()

    tile.TileContext._drain_and_barrier = _compat_drain_and_barrier
    tile.TileContext._bary_compat = True


_install_walrus_compat()

F32 = mybir.dt.float32

N_NODES = 64
M_QUERY = 2097152
N_CORES = 8
QC = M_QUERY // N_CORES      # 262144 queries per core
NG = 8                       # query groups packed into K=128
NP = N_NODES // 2            # 32 pairs (16 per half)
GQ8 = QC // NG               # 32768 queries per group per core
FCHUNK = 2048                # free-dim chunk width

# ---------------------------------------------------------------------------
# Custom DVE ops: q2-product fused with approx-reciprocal seed / NR steps.
# Registered at runtime into concourse.dve_ops.
# ---------------------------------------------------------------------------

_OPS = {}


def _register_custom_ops():
    if _OPS:
        return _OPS
    import concourse.dve_ops as dve_ops
    from concourse.dve_spec import Spec, Src0, Src1, C0, C1, C2, One, Bin, AluOp
    from concourse.dve_table_gen import dve_ver_for

    ver = dve_ver_for("TRN2")

    def _np_f32(v):
        return np.asarray(v, np.float32)

    # out = y1 where q=(x+c0)(x+c1); y0 = bitcast(~bits(q))*c2 ; y1 = y0*(2-q*y0)
    def _ref_seed(in0, in1, c0, c1, c2):
        x = _np_f32(in0)
        q = _np_f32(_np_f32(x + c0) * _np_f32(x + c1))
        nq = (~q.view(np.int32)).view(np.float32)
        y0 = _np_f32(nq * np.float32(c2))
        return _np_f32(y0 * _np_f32(np.float32(2.0) - _np_f32(q * y0)))

    # out = y*(2-q*y) with y = in1, q = (x+c0)(x+c1)
    def _ref_nr(in0, in1, c0, c1, c2):
        x = _np_f32(in0)
        y = _np_f32(in1)
        q = _np_f32(_np_f32(x + c0) * _np_f32(x + c1))
        return _np_f32(y * _np_f32(np.float32(2.0) - _np_f32(q * y)))

    a0 = Src0 + C0
    a1 = Src0 + C1
    q = a0 * a1
    nq = Bin(AluOp.BITWISE_NOT, q, q)
    y0 = nq * C2
    two = One + One  # stream-invariant, hoisted
    y1 = y0 * (two - q * y0)
    seed_spec = Spec(body=y1, reference=_ref_seed)

    qn = (Src0 + C0) * (Src0 + C1)
    nr_spec = Spec(body=Src1 * ((One + One) - qn * Src1), reference=_ref_nr)

    def _make(name, spec):
        if name not in dve_ops._SUB_OPCODE_FOR_NAME:
            dve_ops._SUB_OPCODE_FOR_NAME[name] = (
                dve_ops._CUSTOM_DVE_ROW_BASE + len(dve_ops.OPS)
            )
            dve_ops.OPS.append(None)  # placeholder keeps row indices stable
        row = dve_ops._SUB_OPCODE_FOR_NAME[name] - dve_ops._CUSTOM_DVE_ROW_BASE
        op = dve_ops.DveOp(name, spec, subdim=False, uops_sha={})
        try:
            op.compile(ver)
        except ValueError as e:
            m = re.search(r"drifted \((\w+): ([0-9a-f]+)", str(e))
            if not m:
                raise
            op = dve_ops.DveOp(name, spec, subdim=False,
                               uops_sha={m.group(1): m.group(2)})
        op.compile(ver)
        dve_ops.OPS[row] = op
        dve_ops.CUSTOM_DVE_SPECS[name] = spec
        return op

    seed_op = _make("BARY_PAIRQ_SEED", seed_spec)
    nr_op = _make("BARY_PAIRQ_NR", nr_spec)
    assert max(dve_ops._SUB_OPCODE_FOR_NAME.values()) < 0x20

    _OPS["seed"] = seed_op
    _OPS["nr"] = nr_op
    return _OPS


# ---------------------------------------------------------------------------
# Host-side coefficient preparation (float64, then rounded to f32)
# ---------------------------------------------------------------------------


def _host_coeffs(xi, fi, wi):
    xi = np.asarray(xi, np.float32)
    fi = np.asarray(fi, np.float32)
    wi = np.asarray(wi, np.float32)
    fw = (fi * wi).astype(np.float32)  # matches reference's f32 fi*wi

    # pair node P with node P+32: separation >= ~1.0 for Chebyshev nodes,
    # which keeps the (alpha*x + beta) linear form well-conditioned near roots
    a = xi[:32].astype(np.float64)
    b = xi[32:].astype(np.float64)
    fwa = fw[:32].astype(np.float64)
    fwb = fw[32:].astype(np.float64)
    wa = wi[:32].astype(np.float64)
    wb = wi[32:].astype(np.float64)

    alpha_f = (fwa + fwb).astype(np.float32)
    beta_f = (-(fwa * b + fwb * a)).astype(np.float32)
    alpha_w = (wa + wb).astype(np.float32)
    beta_w = (-(wa * b + wb * a)).astype(np.float32)

    # per-partition root constants; partition r = 8*p + g (pair-in-half p,
    # group g); column/stationary-half h selects pair P = 16*h + p.
    negA = np.zeros((128, 2), np.float32)
    negB = np.zeros((128, 2), np.float32)
    negM = np.zeros((128, 2), np.float32)   # -(a+b)/2
    negD2 = np.zeros((128, 2), np.float32)  # -((a-b)/2)^2
    S = np.zeros((128, 64), np.float32)
    for h in range(2):
        for p in range(16):
            P = 16 * h + p
            for g in range(8):
                r = 8 * p + g
                negA[r, h] = -xi[P]
                negB[r, h] = -xi[P + 32]
                negM[r, h] = np.float32(-(a[P] + b[P]) / 2.0)
                negD2[r, h] = np.float32(-((a[P] - b[P]) / 2.0) ** 2)
                S[r, 32 * h + 4 * g + 0] = alpha_f[P]
                S[r, 32 * h + 4 * g + 1] = beta_f[P]
                S[r, 32 * h + 4 * g + 2] = alpha_w[P]
                S[r, 32 * h + 4 * g + 3] = beta_w[P]
    return negA, negB, negM, negD2, S


# ---------------------------------------------------------------------------
# Bass module builder (parametric in group-queries / chunk size for testing)
# ---------------------------------------------------------------------------


def build_nc(gq=GQ8, fchunk=FCHUNK, mode="actq2", walrus_compat=True):
    """gq = queries per group (8 groups per core); fchunk = chunk width.
    mode: "actq2" (ScalarE computes q2, DVE does fast+NR recip) or
          "dve3" (3 fused custom DVE ops per half)."""
    ops = _register_custom_ops()
    assert gq % fchunk == 0
    nchunks = gq // fchunk
    nmm = fchunk // 512
    assert fchunk % 512 == 0
    assert nchunks * 8 <= 128

    nc = bass.Bass()
    x_in = nc.dram_tensor("x", [8, gq], F32, kind="ExternalInput")
    negA_in = nc.dram_tensor("negA", [128, 2], F32, kind="ExternalInput")
    negB_in = nc.dram_tensor("negB", [128, 2], F32, kind="ExternalInput")
    negM_in = nc.dram_tensor("negM", [128, 2], F32, kind="ExternalInput")
    negD2_in = nc.dram_tensor("negD2", [128, 2], F32, kind="ExternalInput")
    S_in = nc.dram_tensor("S", [128, 64], F32, kind="ExternalInput")
    out_d = nc.dram_tensor("out", [8, gq], F32, kind="ExternalOutput")

    npart = nchunks * 8

    with tile.TileContext(nc) as tc:
        tc._bary_walrus_compat = walrus_compat
        with (
            tc.tile_pool(name="const", bufs=1) as cpool,
            tc.tile_pool(name="xrep", bufs=3) as xpool,
            tc.tile_pool(name="ys", bufs=4) as ypool,
            tc.tile_pool(name="psum", bufs=2, space="PSUM") as ppool,
            tc.tile_pool(name="pstage", bufs=2) as pspool,
            tc.tile_pool(name="stage", bufs=1) as spool,
            tc.tile_pool(name="epi", bufs=2) as epool,
        ):
            negA_t = cpool.tile([128, 2], F32)
            nc.sync.dma_start(negA_t[:, :], negA_in[:, :])
            negB_t = cpool.tile([128, 2], F32)
            nc.sync.dma_start(negB_t[:, :], negB_in[:, :])
            negM_t = cpool.tile([128, 2], F32)
            nc.sync.dma_start(negM_t[:, :], negM_in[:, :])
            negD2_t = cpool.tile([128, 2], F32)
            nc.sync.dma_start(negD2_t[:, :], negD2_in[:, :])
            S_t = cpool.tile([128, 64], F32)
            nc.sync.dma_start(S_t[:, :], S_in[:, :])

            # uber staging: row 8c+g; quantity t at cols [t*fchunk,(t+1)*fchunk)
            uber = spool.tile([npart, 4 * fchunk], F32)
            ub = uber[:, :]
            pstride = ub.ap[0][0]

            for c in range(nchunks):
                xr = xpool.tile([128, fchunk], F32, tag="xr")
                src = x_in[:, c * fchunk:(c + 1) * fchunk].partition_broadcast(16)
                nc.sync.dma_start(xr[:, :], src)

                psum_t = ppool.tile([32, fchunk], F32, tag="ps")
                for h in range(2):
                    if mode == "actq2":
                        u = ypool.tile([128, fchunk], F32, tag="y")
                        nc.scalar.activation(
                            u[:, :], xr[:, :],
                            mybir.ActivationFunctionType.Square,
                            bias=negM_t[:, h:h + 1],
                        )
                        q2 = ypool.tile([128, fchunk], F32, tag="y")
                        nc.scalar.activation(
                            q2[:, :], u[:, :],
                            mybir.ActivationFunctionType.Identity,
                            bias=negD2_t[:, h:h + 1],
                        )
                        y1 = ypool.tile([128, fchunk], F32, tag="y")
                        nc.vector.reciprocal_approx_fast(y1[:, :], q2[:, :])
                        # NR step against the exactly-computed q=(x-a)(x-b):
                        # fixes the (x-m)^2-d2 cancellation near roots
                        y3 = ypool.tile([128, fchunk], F32, tag="y")
                        nc.vector._custom_dve(
                            ops["nr"], out=y3[:, :], in0=xr[:, :], in1=y1[:, :],
                            s0=negA_t[:, h:h + 1], s1=negB_t[:, h:h + 1],
                        )
                    else:
                        y1 = ypool.tile([128, fchunk], F32, tag="y")
                        nc.vector._custom_dve(
                            ops["seed"], out=y1[:, :], in0=xr[:, :],
                            s0=negA_t[:, h:h + 1], s1=negB_t[:, h:h + 1],
                            imm2=-4.0 / 17.0,
                        )
                        y2 = ypool.tile([128, fchunk], F32, tag="y")
                        nc.vector._custom_dve(
                            ops["nr"], out=y2[:, :], in0=xr[:, :], in1=y1[:, :],
                            s0=negA_t[:, h:h + 1], s1=negB_t[:, h:h + 1],
                        )
                        y3 = ypool.tile([128, fchunk], F32, tag="y")
                        nc.vector._custom_dve(
                            ops["nr"], out=y3[:, :], in0=xr[:, :], in1=y2[:, :],
                            s0=negA_t[:, h:h + 1], s1=negB_t[:, h:h + 1],
                        )
                    for k in range(nmm):
                        nc.tensor.matmul(
                            psum_t[:, 512 * k:512 * (k + 1)],
                            S_t[:, 32 * h:32 * (h + 1)],
                            y3[:, 512 * k:512 * (k + 1)],
                            start=(h == 0), stop=(h == 1),
                            skip_group_check=True,
                        )

                # ACT: psum -> sbuf (partition-preserving)
                ps_sb = pspool.tile([32, fchunk], F32, tag="pssb")
                nc.scalar.copy(ps_sb[:, :], psum_t[:, :])

                # DMA re-partition: ps_sb[4g+t, 512k+cc] -> uber[8c+g, t*fchunk+512k+cc]
                dst_ap = bass.AP(
                    tensor=ub.tensor,
                    offset=ub.offset + (8 * c) * pstride,
                    ap=[[pstride, 8], [fchunk, 4], [1, fchunk]],
                )
                eng = nc.sync if c % 2 == 0 else nc.gpsimd
                eng.dma_start(dst_ap, ps_sb[:, :])

            # epilogue over the whole core's queries, in 512-wide slices
            ES = 512
            nslice = fchunk // ES
            for s in range(nslice):
                xe = epool.tile([npart, ES], F32, tag="xe")
                x_ap = bass.AP(
                    tensor=x_in[:, :].tensor, offset=s * ES,
                    ap=[[fchunk, nchunks], [gq, 8], [1, ES]],
                )
                nc.sync.dma_start(xe[:, :], x_ap)

                sl = slice(s * ES, (s + 1) * ES)
                A = uber[:, 0 * fchunk:1 * fchunk][:, sl]
                B = uber[:, 1 * fchunk:2 * fchunk][:, sl]
                C = uber[:, 2 * fchunk:3 * fchunk][:, sl]
                D = uber[:, 3 * fchunk:4 * fchunk][:, sl]

                t1 = epool.tile([npart, ES], F32, tag="t1")
                nc.vector.tensor_mul(t1[:, :], xe[:, :], A)
                numer = epool.tile([npart, ES], F32, tag="numer")
                nc.vector.tensor_add(numer[:, :], t1[:, :], B)
                t2 = epool.tile([npart, ES], F32, tag="t2")
                nc.vector.tensor_mul(t2[:, :], xe[:, :], C)
                denom = epool.tile([npart, ES], F32, tag="denom")
                nc.vector.tensor_add(denom[:, :], t2[:, :], D)

                scratch = epool.tile([npart, ES], F32, tag="scr")
                rd = epool.tile([npart, ES], F32, tag="rd")
                nc.vector.reciprocal_approx_accurate(rd[:, :], denom[:, :], scratch[:, :])
                o = epool.tile([npart, ES], F32, tag="o")
                nc.vector.tensor_mul(o[:, :], numer[:, :], rd[:, :])

                out_ap = bass.AP(
                    tensor=out_d[:, :].tensor, offset=s * ES,
                    ap=[[fchunk, nchunks], [gq, 8], [1, ES]],
                )
                nc.sync.dma_start(out_ap, o[:, :])
    if walrus_compat:
        _split_multiwait(nc)
    # populate .instr bytes for InstISA subclasses (custom DVE ops) — raw
    # Bass skips this pass and this walrus then rejects the empty encoding
    mybir.codegen_inst_isa_subclasses(nc)
    return nc


def _split_multiwait(nc):
    """This walrus accepts at most one sync-wait per instruction; hoist
    extras onto same-engine NOPs inserted just before the instruction."""
    import bass_rust
    from concourse.engine_type import EngineType

    ctr = [0]
    for fn in nc.m.functions:
        for bb in fn.blocks:
            il = bb.instructions
            out = []
            changed = False
            for inst in il:
                si = inst.sync_info
                waits = list(si.on_wait) if si is not None else []
                if len(waits) > 1:
                    changed = True
                    for w in waits[:-1]:
                        ctr[0] += 1
                        nop = mybir.InstNoOp(
                            name=f"mwsplit-{ctr[0]}", engine=inst.engine,
                            ins=[], outs=[],
                        )
                        nop.sync_info = bass_rust.SyncInfo(
                            on_wait=[w], on_update=[])
                        out.append(nop)
                    inst.sync_info = bass_rust.SyncInfo(
                        on_wait=[waits[-1]], on_update=list(si.on_update))
                out.append(inst)
            if changed:
                bb.instructions = out


# ---------------------------------------------------------------------------
# Host entry point
# ---------------------------------------------------------------------------

_NC_CACHE = {}


def _get_nc():
    if "nc" not in _NC_CACHE:
        _NC_CACHE["nc"] = build_nc()
    return _NC_CACHE["nc"]


def _fixup_nonfinite(out, x, xi, fi, wi):
    """Recompute non-finite outputs (exact hits -> NaN; denom==0 -> NaN)
    with a faithful f32 reference formula on the host."""
    bad = ~np.isfinite(out)
    # reference legitimately produces +-inf where its f32 denom rounds to 0;
    # our kernel produces NaN there (approx-recip of 0). Recompute every
    # non-finite lane with the straight f32 formula.
    if not bad.any():
        return out
    idx = np.nonzero(bad)[0]
    xb = x[idx].astype(np.float32)
    c = (xb[:, None] - xi[None, :]).astype(np.float32)
    z = c == 0.0
    cs = np.where(z, np.float32(1.0), c)
    r = (np.float32(1.0) / cs).astype(np.float32)
    fw = (fi * wi).astype(np.float32)
    numer = (r * fw[None, :]).sum(axis=1, dtype=np.float32)
    denom = (r * wi[None, :]).sum(axis=1, dtype=np.float32)
    vals = (numer / denom).astype(np.float32)
    hit = z.any(axis=1)
    if hit.any():
        vals[hit] = fi[z.argmax(axis=1)[hit]]
    out = out.copy()
    out[idx] = vals
    return out


def kernel(x, xi, fi, wi, _trace=False):
    x = np.asarray(x, np.float32)
    xi = np.asarray(xi, np.float32)
    fi = np.asarray(fi, np.float32)
    wi = np.asarray(wi, np.float32)

    negA, negB, negM, negD2, S = _host_coeffs(xi, fi, wi)
    nc = _get_nc()

    in_maps = []
    for cidx in range(N_CORES):
        xc = x[cidx * QC:(cidx + 1) * QC].reshape(NG, GQ8)
        in_maps.append({"x": xc, "negA": negA, "negB": negB,
                        "negM": negM, "negD2": negD2, "S": S})

    res = run_bass_kernel_spmd(
        nc, in_maps, core_ids=list(range(N_CORES)), trace=_trace,
    )
    out = np.concatenate([r["out"].reshape(-1) for r in res.results])
    out = _fixup_nonfinite(out, x, xi, fi, wi)
    if _trace:
        kernel._last_exec_time_ns = res.exec_time_ns
        kernel._last_results = res
    return out


# revision 24
# speedup vs baseline: 1.1933x; 1.1933x over previous
"""Barycentric interpolation kernel for Trainium2 (8 NeuronCores, SPMD).

Problem: out[m] = (sum_k c[m,k]*fi[k]*wi[k]) / (sum_k c[m,k]*wi[k]),
c[m,k] = 1/(x[m]-xi[k]) with exact-hit override, N=64 nodes, M=2^21 queries.

Strategy (per core, data-parallel over m):
  * Nodes are grouped into 32 adjacent PAIRS. For pair p with roots (a,b):
      1/(x-a)/(x-b) = r2 = 1/q2,  q2 = (x-a)(x-b)
      fw_a/(x-a) + fw_b/(x-b) = (alpha_f*x + beta_f) * r2
    so numer = x*A + B, denom = x*C + D where
      A = sum_p alpha_f[p]*r2_p   (and B, C, D similarly)
    are computed by the TensorEngine from r2 rows only.
  * r2 is produced by 3 fused custom DVE ops (product + flip-seed + 3
    Newton-Raphson steps total, ~1-2 ulp).
  * Work layout: rows = 32 pairs x 4 query groups (K=128), queries along
    the free dim; x is broadcast into this layout by DMA.
  * PE matmul with a block-diagonal [128,16] stationary yields
    A,B,C,D per query group; ScalarE stages PSUM->SBUF; DVE epilogue does
    numer/denom with the stock approx-reciprocal (accurate variant).
  * Exact node hits (q2 == +-0) propagate NaN through the pipeline and are
    fixed up on the host with a faithful f32 formula.
"""

import os
import re
import sys
import types
import numpy as np

import concourse.bass as bass
import concourse.mybir as mybir
import concourse.tile as tile
from concourse.bass_utils import run_bass_kernel_spmd


def _ensure_ntff_hook():
    """The agent image's antenv lacks axon_hooks; synthesize it so
    run_bass_kernel_spmd(trace=True) can capture NTFF profiles."""
    try:
        from antenv.axon_hooks import get_axon_ntff_profile_hook  # noqa: F401
        return
    except ImportError:
        pass
    try:
        import antenv
        from trn_agent_boot.trn_boot import _ntff_profile_via_ctypes
        mod = types.ModuleType("antenv.axon_hooks")
        _state = {"hook": None}
        mod.set_axon_ntff_profile_hook = lambda h: _state.__setitem__("hook", h)
        mod.get_axon_ntff_profile_hook = lambda: _state["hook"]
        sys.modules["antenv.axon_hooks"] = mod
        antenv.axon_hooks = mod
        so_path = "/opt/axon/libaxon_pjrt.so"
        if os.path.exists(so_path):
            mod.set_axon_ntff_profile_hook(_ntff_profile_via_ctypes(so_path))
    except Exception:
        pass


_ensure_ntff_hook()


def _install_walrus_compat():
    """This image's walrus predates two concourse behaviors:
    1. Multi-wait drains ("Too many sync wait commands"): the Tile kernel-tail
       drain carries one sync-wait per tracked proc; this walrus's CTRL
       encoding takes at most one. Chunk the waits across drains and skip
       the EVENT_SEMAPHORE_RANGE_CLEAR-based sem recycling (also unsupported).
    2. InstISA subclasses (custom DVE ops) need .instr bytes populated
       client-side via codegen_inst_isa_subclasses ("ISA wrong length").
    """
    import bass_rust
    from concourse.vector_clock import ScopedClock

    if getattr(tile.TileContext, "_bary_compat", False):
        return

    def _compat_drain_and_barrier(self, tick_clock, wait_clock):
        drain_inst = self.nc.sync.drain()
        wait_clock.add_sem_waits(
            drain_inst.ins, ScopedClock({None: tick_clock.global_clock})
        )
        d = drain_inst.ins
        waits = list(d.sync_info.on_wait)
        if len(waits) > 1 and getattr(self, "_bary_walrus_compat", True):
            d.sync_info = bass_rust.SyncInfo(on_wait=waits[:1], on_update=[])
            for w in waits[1:]:
                d2 = self.nc.sync.drain()
                d2.ins.sync_info = bass_rust.SyncInfo(on_wait=[w], on_update=[])
        self.nc.all_engine_barrier()
        popped = self.nc._tile_sem_poison_stack.pop()
        assert popped is self._sem_poison
        self.nc.all_engine_barrier()

    tile.TileContext._drain_and_barrier = _compat_drain_and_barrier
    tile.TileContext._bary_compat = True


_install_walrus_compat()

F32 = mybir.dt.float32

N_NODES = 64
M_QUERY = 2097152
N_CORES = 8
QC = M_QUERY // N_CORES      # 262144 queries per core
NG = 8                       # query groups packed into K=128
NP = N_NODES // 2            # 32 pairs (16 per half)
GQ8 = QC // NG               # 32768 queries per group per core
FCHUNK = 2048                # free-dim chunk width

# ---------------------------------------------------------------------------
# Custom DVE ops: q2-product fused with approx-reciprocal seed / NR steps.
# Registered at runtime into concourse.dve_ops.
# ---------------------------------------------------------------------------

_OPS = {}


def _register_custom_ops():
    if _OPS:
        return _OPS
    import concourse.dve_ops as dve_ops
    from concourse.dve_spec import Spec, Src0, Src1, C0, C1, C2, One, Bin, AluOp
    from concourse.dve_table_gen import dve_ver_for

    ver = dve_ver_for("TRN2")

    def _np_f32(v):
        return np.asarray(v, np.float32)

    # out = y1 where q=(x+c0)(x+c1); y0 = bitcast(~bits(q))*c2 ; y1 = y0*(2-q*y0)
    def _ref_seed(in0, in1, c0, c1, c2):
        x = _np_f32(in0)
        q = _np_f32(_np_f32(x + c0) * _np_f32(x + c1))
        nq = (~q.view(np.int32)).view(np.float32)
        y0 = _np_f32(nq * np.float32(c2))
        return _np_f32(y0 * _np_f32(np.float32(2.0) - _np_f32(q * y0)))

    # out = y*(2-q*y) with y = in1, q = (x+c0)(x+c1)
    def _ref_nr(in0, in1, c0, c1, c2):
        x = _np_f32(in0)
        y = _np_f32(in1)
        q = _np_f32(_np_f32(x + c0) * _np_f32(x + c1))
        return _np_f32(y * _np_f32(np.float32(2.0) - _np_f32(q * y)))

    a0 = Src0 + C0
    a1 = Src0 + C1
    q = a0 * a1
    nq = Bin(AluOp.BITWISE_NOT, q, q)
    y0 = nq * C2
    two = One + One  # stream-invariant, hoisted
    y1 = y0 * (two - q * y0)
    seed_spec = Spec(body=y1, reference=_ref_seed)

    qn = (Src0 + C0) * (Src0 + C1)
    nr_spec = Spec(body=Src1 * ((One + One) - qn * Src1), reference=_ref_nr)

    def _make(name, spec):
        if name not in dve_ops._SUB_OPCODE_FOR_NAME:
            dve_ops._SUB_OPCODE_FOR_NAME[name] = (
                dve_ops._CUSTOM_DVE_ROW_BASE + len(dve_ops.OPS)
            )
            dve_ops.OPS.append(None)  # placeholder keeps row indices stable
        row = dve_ops._SUB_OPCODE_FOR_NAME[name] - dve_ops._CUSTOM_DVE_ROW_BASE
        op = dve_ops.DveOp(name, spec, subdim=False, uops_sha={})
        try:
            op.compile(ver)
        except ValueError as e:
            m = re.search(r"drifted \((\w+): ([0-9a-f]+)", str(e))
            if not m:
                raise
            op = dve_ops.DveOp(name, spec, subdim=False,
                               uops_sha={m.group(1): m.group(2)})
        op.compile(ver)
        dve_ops.OPS[row] = op
        dve_ops.CUSTOM_DVE_SPECS[name] = spec
        return op

    seed_op = _make("BARY_PAIRQ_SEED", seed_spec)
    nr_op = _make("BARY_PAIRQ_NR", nr_spec)
    assert max(dve_ops._SUB_OPCODE_FOR_NAME.values()) < 0x20

    _OPS["seed"] = seed_op
    _OPS["nr"] = nr_op
    return _OPS


# ---------------------------------------------------------------------------
# Host-side coefficient preparation (float64, then rounded to f32)
# ---------------------------------------------------------------------------


def _host_coeffs(xi, fi, wi):
    xi = np.asarray(xi, np.float32)
    fi = np.asarray(fi, np.float32)
    wi = np.asarray(wi, np.float32)
    fw = (fi * wi).astype(np.float32)  # matches reference's f32 fi*wi

    # pair node P with node P+32: separation >= ~1.0 for Chebyshev nodes,
    # which keeps the (alpha*x + beta) linear form well-conditioned near roots
    a = xi[:32].astype(np.float64)
    b = xi[32:].astype(np.float64)
    fwa = fw[:32].astype(np.float64)
    fwb = fw[32:].astype(np.float64)
    wa = wi[:32].astype(np.float64)
    wb = wi[32:].astype(np.float64)

    alpha_f = (fwa + fwb).astype(np.float32)
    beta_f = (-(fwa * b + fwb * a)).astype(np.float32)
    alpha_w = (wa + wb).astype(np.float32)
    beta_w = (-(wa * b + wb * a)).astype(np.float32)

    # per-partition root constants; partition r = 8*p + g (pair-in-half p,
    # group g); column/stationary-half h selects pair P = 16*h + p.
    negA = np.zeros((128, 2), np.float32)
    negB = np.zeros((128, 2), np.float32)
    negM = np.zeros((128, 2), np.float32)   # -(a+b)/2
    negD2 = np.zeros((128, 2), np.float32)  # -((a-b)/2)^2
    S = np.zeros((128, 64), np.float32)
    for h in range(2):
        for p in range(16):
            P = 16 * h + p
            for g in range(8):
                r = 8 * p + g
                negA[r, h] = -xi[P]
                negB[r, h] = -xi[P + 32]
                negM[r, h] = np.float32(-(a[P] + b[P]) / 2.0)
                negD2[r, h] = np.float32(-((a[P] - b[P]) / 2.0) ** 2)
                S[r, 32 * h + 4 * g + 0] = alpha_f[P]
                S[r, 32 * h + 4 * g + 1] = beta_f[P]
                S[r, 32 * h + 4 * g + 2] = alpha_w[P]
                S[r, 32 * h + 4 * g + 3] = beta_w[P]
    return negA, negB, negM, negD2, S


# ---------------------------------------------------------------------------
# Bass module builder (parametric in group-queries / chunk size for testing)
# ---------------------------------------------------------------------------


def build_nc(gq=GQ8, fchunk=FCHUNK, mode="actq2", walrus_compat=True):
    """gq = queries per group (8 groups per core); fchunk = chunk width.
    mode: "actq2" (ScalarE computes q2, DVE does fast+NR recip) or
          "dve3" (3 fused custom DVE ops per half)."""
    ops = _register_custom_ops()
    assert gq % fchunk == 0
    nchunks = gq // fchunk
    nmm = fchunk // 512
    assert fchunk % 512 == 0
    assert nchunks * 8 <= 128

    nc = bass.Bass()
    x_in = nc.dram_tensor("x", [8, gq], F32, kind="ExternalInput")
    negA_in = nc.dram_tensor("negA", [128, 2], F32, kind="ExternalInput")
    negB_in = nc.dram_tensor("negB", [128, 2], F32, kind="ExternalInput")
    negM_in = nc.dram_tensor("negM", [128, 2], F32, kind="ExternalInput")
    negD2_in = nc.dram_tensor("negD2", [128, 2], F32, kind="ExternalInput")
    S_in = nc.dram_tensor("S", [128, 64], F32, kind="ExternalInput")
    out_d = nc.dram_tensor("out", [8, gq], F32, kind="ExternalOutput")

    npart = nchunks * 8

    with tile.TileContext(nc) as tc:
        tc._bary_walrus_compat = walrus_compat
        with (
            tc.tile_pool(name="const", bufs=1) as cpool,
            tc.tile_pool(name="xrep", bufs=3) as xpool,
            tc.tile_pool(name="ys", bufs=2) as ypool,
            tc.tile_pool(name="psum", bufs=2, space="PSUM") as ppool,
            tc.tile_pool(name="pstage", bufs=2) as pspool,
            tc.tile_pool(name="stage", bufs=1) as spool,
            tc.tile_pool(name="epi", bufs=1) as epool,
        ):
            negA_t = cpool.tile([128, 2], F32)
            nc.sync.dma_start(negA_t[:, :], negA_in[:, :])
            negB_t = cpool.tile([128, 2], F32)
            nc.sync.dma_start(negB_t[:, :], negB_in[:, :])
            negM_t = cpool.tile([128, 2], F32)
            nc.sync.dma_start(negM_t[:, :], negM_in[:, :])
            negD2_t = cpool.tile([128, 2], F32)
            nc.sync.dma_start(negD2_t[:, :], negD2_in[:, :])
            S_t = cpool.tile([128, 64], F32)
            nc.sync.dma_start(S_t[:, :], S_in[:, :])

            # uber staging: row 8c+g; quantity t at cols [t*fchunk,(t+1)*fchunk)
            uber = spool.tile([npart, 4 * fchunk], F32)
            ub = uber[:, :]
            pstride = ub.ap[0][0]

            for c in range(nchunks):
                xr = xpool.tile([128, fchunk], F32, tag="xr")
                src = x_in[:, c * fchunk:(c + 1) * fchunk].partition_broadcast(16)
                nc.sync.dma_start(xr[:, :], src)

                psum_t = ppool.tile([32, fchunk], F32, tag="ps")
                for h in range(2):
                    if mode == "actq2":
                        u = ypool.tile([128, fchunk], F32, tag="u")
                        nc.scalar.activation(
                            u[:, :], xr[:, :],
                            mybir.ActivationFunctionType.Square,
                            bias=negM_t[:, h:h + 1],
                        )
                        q2 = ypool.tile([128, fchunk], F32, tag="q2")
                        nc.scalar.activation(
                            q2[:, :], u[:, :],
                            mybir.ActivationFunctionType.Identity,
                            bias=negD2_t[:, h:h + 1],
                        )
                        y1 = ypool.tile([128, fchunk], F32, tag="y1")
                        nc.vector.reciprocal_approx_fast(y1[:, :], q2[:, :])
                        # NR step against the exactly-computed q=(x-a)(x-b):
                        # fixes the (x-m)^2-d2 cancellation near roots
                        y3 = ypool.tile([128, fchunk], F32, tag="y3")
                        nc.vector._custom_dve(
                            ops["nr"], out=y3[:, :], in0=xr[:, :], in1=y1[:, :],
                            s0=negA_t[:, h:h + 1], s1=negB_t[:, h:h + 1],
                        )
                    else:
                        y1 = ypool.tile([128, fchunk], F32, tag="y")
                        nc.vector._custom_dve(
                            ops["seed"], out=y1[:, :], in0=xr[:, :],
                            s0=negA_t[:, h:h + 1], s1=negB_t[:, h:h + 1],
                            imm2=-4.0 / 17.0,
                        )
                        y2 = ypool.tile([128, fchunk], F32, tag="y")
                        nc.vector._custom_dve(
                            ops["nr"], out=y2[:, :], in0=xr[:, :], in1=y1[:, :],
                            s0=negA_t[:, h:h + 1], s1=negB_t[:, h:h + 1],
                        )
                        y3 = ypool.tile([128, fchunk], F32, tag="y")
                        nc.vector._custom_dve(
                            ops["nr"], out=y3[:, :], in0=xr[:, :], in1=y2[:, :],
                            s0=negA_t[:, h:h + 1], s1=negB_t[:, h:h + 1],
                        )
                    for k in range(nmm):
                        nc.tensor.matmul(
                            psum_t[:, 512 * k:512 * (k + 1)],
                            S_t[:, 32 * h:32 * (h + 1)],
                            y3[:, 512 * k:512 * (k + 1)],
                            start=(h == 0), stop=(h == 1),
                            skip_group_check=True,
                        )

                # ACT: psum -> sbuf (partition-preserving)
                ps_sb = pspool.tile([32, fchunk], F32, tag="pssb")
                nc.scalar.copy(ps_sb[:, :], psum_t[:, :])

                # DMA re-partition: ps_sb[4g+t, 512k+cc] -> uber[8c+g, t*fchunk+512k+cc]
                dst_ap = bass.AP(
                    tensor=ub.tensor,
                    offset=ub.offset + (8 * c) * pstride,
                    ap=[[pstride, 8], [fchunk, 4], [1, fchunk]],
                )
                eng = nc.sync if c % 2 == 0 else nc.gpsimd
                eng.dma_start(dst_ap, ps_sb[:, :])

            # epilogue over the whole core's queries, in 512-wide slices
            ES = 512
            nslice = fchunk // ES
            for s in range(nslice):
                xe = epool.tile([npart, ES], F32, tag="xe")
                x_ap = bass.AP(
                    tensor=x_in[:, :].tensor, offset=s * ES,
                    ap=[[fchunk, nchunks], [gq, 8], [1, ES]],
                )
                nc.sync.dma_start(xe[:, :], x_ap)

                sl = slice(s * ES, (s + 1) * ES)
                A = uber[:, 0 * fchunk:1 * fchunk][:, sl]
                B = uber[:, 1 * fchunk:2 * fchunk][:, sl]
                C = uber[:, 2 * fchunk:3 * fchunk][:, sl]
                D = uber[:, 3 * fchunk:4 * fchunk][:, sl]

                t1 = epool.tile([npart, ES], F32, tag="t1")
                nc.gpsimd.tensor_mul(t1[:, :], xe[:, :], A)
                numer = epool.tile([npart, ES], F32, tag="numer")
                nc.gpsimd.tensor_add(numer[:, :], t1[:, :], B)
                t2 = epool.tile([npart, ES], F32, tag="t2")
                nc.gpsimd.tensor_mul(t2[:, :], xe[:, :], C)
                denom = epool.tile([npart, ES], F32, tag="denom")
                nc.gpsimd.tensor_add(denom[:, :], t2[:, :], D)

                scratch = epool.tile([npart, ES], F32, tag="scr")
                rd = epool.tile([npart, ES], F32, tag="rd")
                nc.vector.reciprocal_approx_accurate(rd[:, :], denom[:, :], scratch[:, :])
                o = epool.tile([npart, ES], F32, tag="o")
                nc.gpsimd.tensor_mul(o[:, :], numer[:, :], rd[:, :])

                out_ap = bass.AP(
                    tensor=out_d[:, :].tensor, offset=s * ES,
                    ap=[[fchunk, nchunks], [gq, 8], [1, ES]],
                )
                nc.sync.dma_start(out_ap, o[:, :])
    if walrus_compat:
        _split_multiwait(nc)
    # populate .instr bytes for InstISA subclasses (custom DVE ops) — raw
    # Bass skips this pass and this walrus then rejects the empty encoding
    mybir.codegen_inst_isa_subclasses(nc)
    return nc


def _split_multiwait(nc):
    """This walrus accepts at most one sync-wait per instruction; hoist
    extras onto same-engine NOPs inserted just before the instruction."""
    import bass_rust
    from concourse.engine_type import EngineType

    ctr = [0]
    for fn in nc.m.functions:
        for bb in fn.blocks:
            il = bb.instructions
            out = []
            changed = False
            for inst in il:
                si = inst.sync_info
                waits = list(si.on_wait) if si is not None else []
                if len(waits) > 1:
                    changed = True
                    for w in waits[:-1]:
                        ctr[0] += 1
                        nop = mybir.InstNoOp(
                            name=f"mwsplit-{ctr[0]}", engine=inst.engine,
                            ins=[], outs=[],
                        )
                        nop.sync_info = bass_rust.SyncInfo(
                            on_wait=[w], on_update=[])
                        out.append(nop)
                    inst.sync_info = bass_rust.SyncInfo(
                        on_wait=[waits[-1]], on_update=list(si.on_update))
                out.append(inst)
            if changed:
                bb.instructions = out


# ---------------------------------------------------------------------------
# Host entry point
# ---------------------------------------------------------------------------

_NC_CACHE = {}


def _get_nc():
    if "nc" not in _NC_CACHE:
        _NC_CACHE["nc"] = build_nc()
    return _NC_CACHE["nc"]


def _fixup_nonfinite(out, x, xi, fi, wi):
    """Recompute non-finite outputs (exact hits -> NaN; denom==0 -> NaN)
    with a faithful f32 reference formula on the host."""
    bad = ~np.isfinite(out)
    # reference legitimately produces +-inf where its f32 denom rounds to 0;
    # our kernel produces NaN there (approx-recip of 0). Recompute every
    # non-finite lane with the straight f32 formula.
    if not bad.any():
        return out
    idx = np.nonzero(bad)[0]
    xb = x[idx].astype(np.float32)
    c = (xb[:, None] - xi[None, :]).astype(np.float32)
    z = c == 0.0
    cs = np.where(z, np.float32(1.0), c)
    r = (np.float32(1.0) / cs).astype(np.float32)
    fw = (fi * wi).astype(np.float32)
    numer = (r * fw[None, :]).sum(axis=1, dtype=np.float32)
    denom = (r * wi[None, :]).sum(axis=1, dtype=np.float32)
    vals = (numer / denom).astype(np.float32)
    hit = z.any(axis=1)
    if hit.any():
        vals[hit] = fi[z.argmax(axis=1)[hit]]
    out = out.copy()
    out[idx] = vals
    return out


def kernel(x, xi, fi, wi, _trace=False):
    x = np.asarray(x, np.float32)
    xi = np.asarray(xi, np.float32)
    fi = np.asarray(fi, np.float32)
    wi = np.asarray(wi, np.float32)

    negA, negB, negM, negD2, S = _host_coeffs(xi, fi, wi)
    nc = _get_nc()

    in_maps = []
    for cidx in range(N_CORES):
        xc = x[cidx * QC:(cidx + 1) * QC].reshape(NG, GQ8)
        in_maps.append({"x": xc, "negA": negA, "negB": negB,
                        "negM": negM, "negD2": negD2, "S": S})

    res = run_bass_kernel_spmd(
        nc, in_maps, core_ids=list(range(N_CORES)), trace=_trace,
    )
    out = np.concatenate([r["out"].reshape(-1) for r in res.results])
    out = _fixup_nonfinite(out, x, xi, fi, wi)
    if _trace:
        kernel._last_exec_time_ns = res.exec_time_ns
        kernel._last_results = res
    return out


# revision 27
# speedup vs baseline: 1.2614x; 1.0571x over previous
"""Barycentric interpolation kernel for Trainium2 (8 NeuronCores, SPMD).

Problem: out[m] = (sum_k c[m,k]*fi[k]*wi[k]) / (sum_k c[m,k]*wi[k]),
c[m,k] = 1/(x[m]-xi[k]) with exact-hit override, N=64 nodes, M=2^21 queries.

Strategy (per core, data-parallel over m):
  * Nodes are grouped into 32 adjacent PAIRS. For pair p with roots (a,b):
      1/(x-a)/(x-b) = r2 = 1/q2,  q2 = (x-a)(x-b)
      fw_a/(x-a) + fw_b/(x-b) = (alpha_f*x + beta_f) * r2
    so numer = x*A + B, denom = x*C + D where
      A = sum_p alpha_f[p]*r2_p   (and B, C, D similarly)
    are computed by the TensorEngine from r2 rows only.
  * r2 is produced by 3 fused custom DVE ops (product + flip-seed + 3
    Newton-Raphson steps total, ~1-2 ulp).
  * Work layout: rows = 32 pairs x 4 query groups (K=128), queries along
    the free dim; x is broadcast into this layout by DMA.
  * PE matmul with a block-diagonal [128,16] stationary yields
    A,B,C,D per query group; ScalarE stages PSUM->SBUF; DVE epilogue does
    numer/denom with the stock approx-reciprocal (accurate variant).
  * Exact node hits (q2 == +-0) propagate NaN through the pipeline and are
    fixed up on the host with a faithful f32 formula.
"""

import os
import re
import sys
import types
import numpy as np

import concourse.bass as bass
import concourse.mybir as mybir
import concourse.tile as tile
from concourse.bass_utils import run_bass_kernel_spmd


def _ensure_ntff_hook():
    """The agent image's antenv lacks axon_hooks; synthesize it so
    run_bass_kernel_spmd(trace=True) can capture NTFF profiles."""
    try:
        from antenv.axon_hooks import get_axon_ntff_profile_hook  # noqa: F401
        return
    except ImportError:
        pass
    try:
        import antenv
        from trn_agent_boot.trn_boot import _ntff_profile_via_ctypes
        mod = types.ModuleType("antenv.axon_hooks")
        _state = {"hook": None}
        mod.set_axon_ntff_profile_hook = lambda h: _state.__setitem__("hook", h)
        mod.get_axon_ntff_profile_hook = lambda: _state["hook"]
        sys.modules["antenv.axon_hooks"] = mod
        antenv.axon_hooks = mod
        so_path = "/opt/axon/libaxon_pjrt.so"
        if os.path.exists(so_path):
            mod.set_axon_ntff_profile_hook(_ntff_profile_via_ctypes(so_path))
    except Exception:
        pass


_ensure_ntff_hook()


def _install_walrus_compat():
    """This image's walrus predates two concourse behaviors:
    1. Multi-wait drains ("Too many sync wait commands"): the Tile kernel-tail
       drain carries one sync-wait per tracked proc; this walrus's CTRL
       encoding takes at most one. Chunk the waits across drains and skip
       the EVENT_SEMAPHORE_RANGE_CLEAR-based sem recycling (also unsupported).
    2. InstISA subclasses (custom DVE ops) need .instr bytes populated
       client-side via codegen_inst_isa_subclasses ("ISA wrong length").
    """
    import bass_rust
    from concourse.vector_clock import ScopedClock

    if getattr(tile.TileContext, "_bary_compat", False):
        return

    def _compat_drain_and_barrier(self, tick_clock, wait_clock):
        drain_inst = self.nc.sync.drain()
        wait_clock.add_sem_waits(
            drain_inst.ins, ScopedClock({None: tick_clock.global_clock})
        )
        d = drain_inst.ins
        waits = list(d.sync_info.on_wait)
        if len(waits) > 1 and getattr(self, "_bary_walrus_compat", True):
            d.sync_info = bass_rust.SyncInfo(on_wait=waits[:1], on_update=[])
            for w in waits[1:]:
                d2 = self.nc.sync.drain()
                d2.ins.sync_info = bass_rust.SyncInfo(on_wait=[w], on_update=[])
        self.nc.all_engine_barrier()
        popped = self.nc._tile_sem_poison_stack.pop()
        assert popped is self._sem_poison
        self.nc.all_engine_barrier()

    tile.TileContext._drain_and_barrier = _compat_drain_and_barrier
    tile.TileContext._bary_compat = True


_install_walrus_compat()

F32 = mybir.dt.float32

N_NODES = 64
M_QUERY = 2097152
N_CORES = 8
QC = M_QUERY // N_CORES      # 262144 queries per core
NG = 8                       # query groups packed into K=128
NP = N_NODES // 2            # 32 pairs (16 per half)
GQ8 = QC // NG               # 32768 queries per group per core
FCHUNK = 2048                # free-dim chunk width

# ---------------------------------------------------------------------------
# Custom DVE ops: q2-product fused with approx-reciprocal seed / NR steps.
# Registered at runtime into concourse.dve_ops.
# ---------------------------------------------------------------------------

_OPS = {}


def _register_custom_ops():
    if _OPS:
        return _OPS
    import concourse.dve_ops as dve_ops
    from concourse.dve_spec import Spec, Src0, Src1, C0, C1, C2, One, Bin, AluOp
    from concourse.dve_table_gen import dve_ver_for

    ver = dve_ver_for("TRN2")

    def _np_f32(v):
        return np.asarray(v, np.float32)

    # out = y1 where q=(x+c0)(x+c1); y0 = bitcast(~bits(q))*c2 ; y1 = y0*(2-q*y0)
    def _ref_seed(in0, in1, c0, c1, c2):
        x = _np_f32(in0)
        q = _np_f32(_np_f32(x + c0) * _np_f32(x + c1))
        nq = (~q.view(np.int32)).view(np.float32)
        y0 = _np_f32(nq * np.float32(c2))
        return _np_f32(y0 * _np_f32(np.float32(2.0) - _np_f32(q * y0)))

    # out = y*(2-q*y) with y = in1, q = (x+c0)(x+c1)
    def _ref_nr(in0, in1, c0, c1, c2):
        x = _np_f32(in0)
        y = _np_f32(in1)
        q = _np_f32(_np_f32(x + c0) * _np_f32(x + c1))
        return _np_f32(y * _np_f32(np.float32(2.0) - _np_f32(q * y)))

    a0 = Src0 + C0
    a1 = Src0 + C1
    q = a0 * a1
    nq = Bin(AluOp.BITWISE_NOT, q, q)
    y0 = nq * C2
    two = One + One  # stream-invariant, hoisted
    y1 = y0 * (two - q * y0)
    seed_spec = Spec(body=y1, reference=_ref_seed)

    qn = (Src0 + C0) * (Src0 + C1)
    nr_spec = Spec(body=Src1 * ((One + One) - qn * Src1), reference=_ref_nr)

    def _make(name, spec):
        if name not in dve_ops._SUB_OPCODE_FOR_NAME:
            dve_ops._SUB_OPCODE_FOR_NAME[name] = (
                dve_ops._CUSTOM_DVE_ROW_BASE + len(dve_ops.OPS)
            )
            dve_ops.OPS.append(None)  # placeholder keeps row indices stable
        row = dve_ops._SUB_OPCODE_FOR_NAME[name] - dve_ops._CUSTOM_DVE_ROW_BASE
        op = dve_ops.DveOp(name, spec, subdim=False, uops_sha={})
        try:
            op.compile(ver)
        except ValueError as e:
            m = re.search(r"drifted \((\w+): ([0-9a-f]+)", str(e))
            if not m:
                raise
            op = dve_ops.DveOp(name, spec, subdim=False,
                               uops_sha={m.group(1): m.group(2)})
        op.compile(ver)
        dve_ops.OPS[row] = op
        dve_ops.CUSTOM_DVE_SPECS[name] = spec
        return op

    # seed from u=(x-m)^2: q = u + c0(-d2); flip-seed + one Chebyshev NR
    def _ref_seed_u(in0, in1, c0, c1, c2):
        u = _np_f32(in0)
        q = _np_f32(u + c0)
        nq = (~q.view(np.int32)).view(np.float32)
        y0 = _np_f32(nq * np.float32(c2))
        return _np_f32(y0 * _np_f32(np.float32(c1) - _np_f32(q * y0)))

    qu = Src0 + C0
    nqu = Bin(AluOp.BITWISE_NOT, qu, qu)
    y0u = nqu * C2
    seedu_spec = Spec(body=y0u * (C1 - qu * y0u), reference=_ref_seed_u)

    seed_op = _make("BARY_PAIRQ_SEED", seed_spec)
    nr_op = _make("BARY_PAIRQ_NR", nr_spec)
    seedu_op = _make("BARY_SEED_U", seedu_spec)
    _OPS["seedu"] = seedu_op
    assert max(dve_ops._SUB_OPCODE_FOR_NAME.values()) < 0x20

    _OPS["seed"] = seed_op
    _OPS["nr"] = nr_op
    return _OPS


# ---------------------------------------------------------------------------
# Host-side coefficient preparation (float64, then rounded to f32)
# ---------------------------------------------------------------------------


def _host_coeffs(xi, fi, wi):
    xi = np.asarray(xi, np.float32)
    fi = np.asarray(fi, np.float32)
    wi = np.asarray(wi, np.float32)
    fw = (fi * wi).astype(np.float32)  # matches reference's f32 fi*wi

    # pair node P with node P+32: separation >= ~1.0 for Chebyshev nodes,
    # which keeps the (alpha*x + beta) linear form well-conditioned near roots
    a = xi[:32].astype(np.float64)
    b = xi[32:].astype(np.float64)
    fwa = fw[:32].astype(np.float64)
    fwb = fw[32:].astype(np.float64)
    wa = wi[:32].astype(np.float64)
    wb = wi[32:].astype(np.float64)

    alpha_f = (fwa + fwb).astype(np.float32)
    beta_f = (-(fwa * b + fwb * a)).astype(np.float32)
    alpha_w = (wa + wb).astype(np.float32)
    beta_w = (-(wa * b + wb * a)).astype(np.float32)

    # per-partition root constants; partition r = 8*p + g (pair-in-half p,
    # group g); column/stationary-half h selects pair P = 16*h + p.
    negA = np.zeros((128, 2), np.float32)
    negB = np.zeros((128, 2), np.float32)
    negM = np.zeros((128, 2), np.float32)   # -(a+b)/2
    negD2 = np.zeros((128, 2), np.float32)  # -((a-b)/2)^2
    S = np.zeros((128, 64), np.float32)
    for h in range(2):
        for p in range(16):
            P = 16 * h + p
            for g in range(8):
                r = 8 * p + g
                negA[r, h] = -xi[P]
                negB[r, h] = -xi[P + 32]
                negM[r, h] = np.float32(-(a[P] + b[P]) / 2.0)
                negD2[r, h] = np.float32(-((a[P] - b[P]) / 2.0) ** 2)
                S[r, 32 * h + 4 * g + 0] = alpha_f[P]
                S[r, 32 * h + 4 * g + 1] = beta_f[P]
                S[r, 32 * h + 4 * g + 2] = alpha_w[P]
                S[r, 32 * h + 4 * g + 3] = beta_w[P]
    return negA, negB, negM, negD2, S


# ---------------------------------------------------------------------------
# Bass module builder (parametric in group-queries / chunk size for testing)
# ---------------------------------------------------------------------------


def build_nc(gq=GQ8, fchunk=FCHUNK, mode="actq2", walrus_compat=True):
    """gq = queries per group (8 groups per core); fchunk = chunk width.
    mode: "actq2" (ScalarE computes q2, DVE does fast+NR recip) or
          "dve3" (3 fused custom DVE ops per half)."""
    ops = _register_custom_ops()
    assert gq % fchunk == 0
    nchunks = gq // fchunk
    nmm = fchunk // 512
    assert fchunk % 512 == 0
    assert nchunks * 8 <= 128

    nc = bass.Bass()
    x_in = nc.dram_tensor("x", [8, gq], F32, kind="ExternalInput")
    negA_in = nc.dram_tensor("negA", [128, 2], F32, kind="ExternalInput")
    negB_in = nc.dram_tensor("negB", [128, 2], F32, kind="ExternalInput")
    negM_in = nc.dram_tensor("negM", [128, 2], F32, kind="ExternalInput")
    negD2_in = nc.dram_tensor("negD2", [128, 2], F32, kind="ExternalInput")
    S_in = nc.dram_tensor("S", [128, 64], F32, kind="ExternalInput")
    out_d = nc.dram_tensor("out", [8, gq], F32, kind="ExternalOutput")

    npart = nchunks * 8

    with tile.TileContext(nc) as tc:
        tc._bary_walrus_compat = walrus_compat
        with (
            tc.tile_pool(name="const", bufs=1) as cpool,
            tc.tile_pool(name="xrep", bufs=4) as xpool,
            tc.tile_pool(name="ys", bufs=2) as ypool,
            tc.tile_pool(name="psum", bufs=2, space="PSUM") as ppool,
            tc.tile_pool(name="pstage", bufs=2) as pspool,
            tc.tile_pool(name="stage", bufs=1) as spool,
            tc.tile_pool(name="epi", bufs=1) as epool,
        ):
            negA_t = cpool.tile([128, 2], F32)
            nc.sync.dma_start(negA_t[:, :], negA_in[:, :])
            negB_t = cpool.tile([128, 2], F32)
            nc.sync.dma_start(negB_t[:, :], negB_in[:, :])
            negM_t = cpool.tile([128, 2], F32)
            nc.sync.dma_start(negM_t[:, :], negM_in[:, :])
            negD2_t = cpool.tile([128, 2], F32)
            nc.sync.dma_start(negD2_t[:, :], negD2_in[:, :])
            S_t = cpool.tile([128, 64], F32)
            nc.sync.dma_start(S_t[:, :], S_in[:, :])

            # uber staging: row 8c+g; quantity t at cols [t*fchunk,(t+1)*fchunk)
            uber = spool.tile([npart, 4 * fchunk], F32)
            ub = uber[:, :]
            pstride = ub.ap[0][0]

            for c in range(nchunks):
                xr = xpool.tile([128, fchunk], F32, tag="xr")
                src = x_in[:, c * fchunk:(c + 1) * fchunk].partition_broadcast(16)
                nc.sync.dma_start(xr[:, :], src)

                psum_t = ppool.tile([32, fchunk], F32, tag="ps")
                for h in range(2):
                    if mode == "actq2s":
                        u = ypool.tile([128, fchunk], F32, tag="u", bufs=3)
                        nc.scalar.activation(
                            u[:, :], xr[:, :],
                            mybir.ActivationFunctionType.Square,
                            bias=negM_t[:, h:h + 1],
                        )
                        y1 = ypool.tile([128, fchunk], F32, tag="y1")
                        nc.vector._custom_dve(
                            ops["seedu"], out=y1[:, :], in0=u[:, :],
                            s0=negD2_t[:, h:h + 1], s1=2.0017324,
                            imm2=-0.23549792,
                        )
                        y3 = ypool.tile([128, fchunk], F32, tag="y3", bufs=3)
                        nc.vector._custom_dve(
                            ops["nr"], out=y3[:, :], in0=xr[:, :], in1=y1[:, :],
                            s0=negA_t[:, h:h + 1], s1=negB_t[:, h:h + 1],
                        )
                    elif mode == "actq2":
                        u = ypool.tile([128, fchunk], F32, tag="u", bufs=3)
                        nc.scalar.activation(
                            u[:, :], xr[:, :],
                            mybir.ActivationFunctionType.Square,
                            bias=negM_t[:, h:h + 1],
                        )
                        q2 = ypool.tile([128, fchunk], F32, tag="q2")
                        nc.scalar.activation(
                            q2[:, :], u[:, :],
                            mybir.ActivationFunctionType.Identity,
                            bias=negD2_t[:, h:h + 1],
                        )
                        y1 = ypool.tile([128, fchunk], F32, tag="y1")
                        nc.vector.reciprocal_approx_fast(y1[:, :], q2[:, :])
                        # NR step against the exactly-computed q=(x-a)(x-b):
                        # fixes the (x-m)^2-d2 cancellation near roots
                        y3 = ypool.tile([128, fchunk], F32, tag="y3", bufs=3)
                        nc.vector._custom_dve(
                            ops["nr"], out=y3[:, :], in0=xr[:, :], in1=y1[:, :],
                            s0=negA_t[:, h:h + 1], s1=negB_t[:, h:h + 1],
                        )
                    else:
                        y1 = ypool.tile([128, fchunk], F32, tag="y")
                        nc.vector._custom_dve(
                            ops["seed"], out=y1[:, :], in0=xr[:, :],
                            s0=negA_t[:, h:h + 1], s1=negB_t[:, h:h + 1],
                            imm2=-4.0 / 17.0,
                        )
                        y2 = ypool.tile([128, fchunk], F32, tag="y")
                        nc.vector._custom_dve(
                            ops["nr"], out=y2[:, :], in0=xr[:, :], in1=y1[:, :],
                            s0=negA_t[:, h:h + 1], s1=negB_t[:, h:h + 1],
                        )
                        y3 = ypool.tile([128, fchunk], F32, tag="y")
                        nc.vector._custom_dve(
                            ops["nr"], out=y3[:, :], in0=xr[:, :], in1=y2[:, :],
                            s0=negA_t[:, h:h + 1], s1=negB_t[:, h:h + 1],
                        )
                    for k in range(nmm):
                        nc.tensor.matmul(
                            psum_t[:, 512 * k:512 * (k + 1)],
                            S_t[:, 32 * h:32 * (h + 1)],
                            y3[:, 512 * k:512 * (k + 1)],
                            start=(h == 0), stop=(h == 1),
                            skip_group_check=True,
                        )

                # ACT: psum -> sbuf (partition-preserving)
                ps_sb = pspool.tile([32, fchunk], F32, tag="pssb")
                nc.scalar.copy(ps_sb[:, :], psum_t[:, :])

                # DMA re-partition: ps_sb[4g+t, 512k+cc] -> uber[8c+g, t*fchunk+512k+cc]
                dst_ap = bass.AP(
                    tensor=ub.tensor,
                    offset=ub.offset + (8 * c) * pstride,
                    ap=[[pstride, 8], [fchunk, 4], [1, fchunk]],
                )
                eng = nc.sync if c % 2 == 0 else nc.gpsimd
                eng.dma_start(dst_ap, ps_sb[:, :])

            # epilogue over the whole core's queries, in 512-wide slices
            ES = 512
            nslice = fchunk // ES
            for s in range(nslice):
                xe = epool.tile([npart, ES], F32, tag="xe")
                x_ap = bass.AP(
                    tensor=x_in[:, :].tensor, offset=s * ES,
                    ap=[[fchunk, nchunks], [gq, 8], [1, ES]],
                )
                nc.sync.dma_start(xe[:, :], x_ap)

                sl = slice(s * ES, (s + 1) * ES)
                A = uber[:, 0 * fchunk:1 * fchunk][:, sl]
                B = uber[:, 1 * fchunk:2 * fchunk][:, sl]
                C = uber[:, 2 * fchunk:3 * fchunk][:, sl]
                D = uber[:, 3 * fchunk:4 * fchunk][:, sl]

                t1 = epool.tile([npart, ES], F32, tag="t1")
                nc.gpsimd.tensor_mul(t1[:, :], xe[:, :], A)
                numer = epool.tile([npart, ES], F32, tag="numer")
                nc.gpsimd.tensor_add(numer[:, :], t1[:, :], B)
                t2 = epool.tile([npart, ES], F32, tag="t2")
                nc.gpsimd.tensor_mul(t2[:, :], xe[:, :], C)
                denom = epool.tile([npart, ES], F32, tag="denom")
                nc.gpsimd.tensor_add(denom[:, :], t2[:, :], D)

                scratch = epool.tile([npart, ES], F32, tag="scr")
                rd = epool.tile([npart, ES], F32, tag="rd")
                nc.vector.reciprocal_approx_accurate(rd[:, :], denom[:, :], scratch[:, :])
                o = epool.tile([npart, ES], F32, tag="o")
                nc.gpsimd.tensor_mul(o[:, :], numer[:, :], rd[:, :])

                out_ap = bass.AP(
                    tensor=out_d[:, :].tensor, offset=s * ES,
                    ap=[[fchunk, nchunks], [gq, 8], [1, ES]],
                )
                nc.sync.dma_start(out_ap, o[:, :])
    if walrus_compat:
        _split_multiwait(nc)
    # populate .instr bytes for InstISA subclasses (custom DVE ops) — raw
    # Bass skips this pass and this walrus then rejects the empty encoding
    mybir.codegen_inst_isa_subclasses(nc)
    return nc


def _split_multiwait(nc):
    """This walrus accepts at most one sync-wait per instruction; hoist
    extras onto same-engine NOPs inserted just before the instruction."""
    import bass_rust
    from concourse.engine_type import EngineType

    ctr = [0]
    for fn in nc.m.functions:
        for bb in fn.blocks:
            il = bb.instructions
            out = []
            changed = False
            for inst in il:
                si = inst.sync_info
                waits = list(si.on_wait) if si is not None else []
                if len(waits) > 1:
                    changed = True
                    for w in waits[:-1]:
                        ctr[0] += 1
                        nop = mybir.InstNoOp(
                            name=f"mwsplit-{ctr[0]}", engine=inst.engine,
                            ins=[], outs=[],
                        )
                        nop.sync_info = bass_rust.SyncInfo(
                            on_wait=[w], on_update=[])
                        out.append(nop)
                    inst.sync_info = bass_rust.SyncInfo(
                        on_wait=[waits[-1]], on_update=list(si.on_update))
                out.append(inst)
            if changed:
                bb.instructions = out


# ---------------------------------------------------------------------------
# Host entry point
# ---------------------------------------------------------------------------

_NC_CACHE = {}


def _get_nc():
    if "nc" not in _NC_CACHE:
        _NC_CACHE["nc"] = build_nc()
    return _NC_CACHE["nc"]


def _fixup_nonfinite(out, x, xi, fi, wi):
    """Recompute non-finite outputs (exact hits -> NaN; denom==0 -> NaN)
    with a faithful f32 reference formula on the host."""
    bad = ~np.isfinite(out)
    # reference legitimately produces +-inf where its f32 denom rounds to 0;
    # our kernel produces NaN there (approx-recip of 0). Recompute every
    # non-finite lane with the straight f32 formula.
    if not bad.any():
        return out
    idx = np.nonzero(bad)[0]
    xb = x[idx].astype(np.float32)
    c = (xb[:, None] - xi[None, :]).astype(np.float32)
    z = c == 0.0
    cs = np.where(z, np.float32(1.0), c)
    r = (np.float32(1.0) / cs).astype(np.float32)
    fw = (fi * wi).astype(np.float32)
    numer = (r * fw[None, :]).sum(axis=1, dtype=np.float32)
    denom = (r * wi[None, :]).sum(axis=1, dtype=np.float32)
    vals = (numer / denom).astype(np.float32)
    hit = z.any(axis=1)
    if hit.any():
        vals[hit] = fi[z.argmax(axis=1)[hit]]
    out = out.copy()
    out[idx] = vals
    return out


def kernel(x, xi, fi, wi, _trace=False):
    x = np.asarray(x, np.float32)
    xi = np.asarray(xi, np.float32)
    fi = np.asarray(fi, np.float32)
    wi = np.asarray(wi, np.float32)

    negA, negB, negM, negD2, S = _host_coeffs(xi, fi, wi)
    nc = _get_nc()

    in_maps = []
    for cidx in range(N_CORES):
        xc = x[cidx * QC:(cidx + 1) * QC].reshape(NG, GQ8)
        in_maps.append({"x": xc, "negA": negA, "negB": negB,
                        "negM": negM, "negD2": negD2, "S": S})

    res = run_bass_kernel_spmd(
        nc, in_maps, core_ids=list(range(N_CORES)), trace=_trace,
    )
    out = np.concatenate([r["out"].reshape(-1) for r in res.results])
    out = _fixup_nonfinite(out, x, xi, fi, wi)
    if _trace:
        kernel._last_exec_time_ns = res.exec_time_ns
        kernel._last_results = res
    return out


# revision 35
# speedup vs baseline: 1.3020x; 1.0321x over previous
"""Barycentric interpolation kernel for Trainium2 (8 NeuronCores, SPMD).

Problem: out[m] = (sum_k c[m,k]*fi[k]*wi[k]) / (sum_k c[m,k]*wi[k]),
c[m,k] = 1/(x[m]-xi[k]) with exact-hit override, N=64 nodes, M=2^21 queries.

Strategy (per core, data-parallel over m):
  * Nodes are paired (P, P+32) — separation >= ~1 keeps the linear forms
    well conditioned. For pair p with roots (a,b):
      r2 = 1/q2, q2 = (x-a)(x-b)
      fw_a/(x-a) + fw_b/(x-b) = (alpha_f*x + beta_f) * r2
    so numer = x*A + B, denom = x*C + D where A = sum_p alpha_f[p]*r2_p
    (B, C, D similarly) come from TensorE matmuls over r2 rows only.
  * Work layout: rows = 16 pairs x 8 query groups (K=128), queries along
    the free dim; x is broadcast into this layout by DMA; the two
    pair-halves accumulate into one PSUM [32, fchunk] tile (M=32).
  * r2 pipeline (mode "actq2"): ScalarE computes u=(x-m)^2 and
    q2=u-d2 (per-partition bias vectors); DVE runs the stock flip-seed
    approx reciprocal then one custom Newton step against the exactly
    computed q=(x-a)(x-b), restoring ~1-2 ulp near roots.
  * ScalarE stages PSUM->SBUF; DMA re-partitions into a [128, 4*fchunk]
    staging tile; GpSimd + DVE do the final (x*A+B)/(x*C+D).
  * Exact node hits (q2 == +-0) propagate NaN through the pipeline and
    are fixed up on the host with a faithful f32 formula (the reference
    itself yields +-inf wherever its f32 denominator rounds to zero, so
    non-finite lanes are recomputed with the straight f32 formula).

Engine budget per core (HW-measured): ScalarE ~153us, DVE ~151us,
TensorE ~123us (fp32 matmul; fp32r is single-pass reduced precision and
cannot carry these 1e17-scale cancelling sums), GpSimd ~30us.
"""

import os
import re
import sys
import types
import numpy as np

import concourse.bass as bass
import concourse.mybir as mybir
import concourse.tile as tile
from concourse.bass_utils import run_bass_kernel_spmd


def _ensure_ntff_hook():
    """The agent image's antenv lacks axon_hooks; synthesize it so
    run_bass_kernel_spmd(trace=True) can capture NTFF profiles."""
    try:
        from antenv.axon_hooks import get_axon_ntff_profile_hook  # noqa: F401
        return
    except ImportError:
        pass
    try:
        import antenv
        from trn_agent_boot.trn_boot import _ntff_profile_via_ctypes
        mod = types.ModuleType("antenv.axon_hooks")
        _state = {"hook": None}
        mod.set_axon_ntff_profile_hook = lambda h: _state.__setitem__("hook", h)
        mod.get_axon_ntff_profile_hook = lambda: _state["hook"]
        sys.modules["antenv.axon_hooks"] = mod
        antenv.axon_hooks = mod
        so_path = "/opt/axon/libaxon_pjrt.so"
        if os.path.exists(so_path):
            mod.set_axon_ntff_profile_hook(_ntff_profile_via_ctypes(so_path))
    except Exception:
        pass


_ensure_ntff_hook()


def _install_walrus_compat():
    """This image's walrus predates two concourse behaviors:
    1. Multi-wait drains ("Too many sync wait commands"): the Tile kernel-tail
       drain carries one sync-wait per tracked proc; this walrus's CTRL
       encoding takes at most one. Chunk the waits across drains and skip
       the EVENT_SEMAPHORE_RANGE_CLEAR-based sem recycling (also unsupported).
    2. InstISA subclasses (custom DVE ops) need .instr bytes populated
       client-side via codegen_inst_isa_subclasses ("ISA wrong length").
    """
    import bass_rust
    from concourse.vector_clock import ScopedClock

    if getattr(tile.TileContext, "_bary_compat", False):
        return

    def _compat_drain_and_barrier(self, tick_clock, wait_clock):
        drain_inst = self.nc.sync.drain()
        wait_clock.add_sem_waits(
            drain_inst.ins, ScopedClock({None: tick_clock.global_clock})
        )
        d = drain_inst.ins
        waits = list(d.sync_info.on_wait)
        if len(waits) > 1 and getattr(self, "_bary_walrus_compat", True):
            d.sync_info = bass_rust.SyncInfo(on_wait=waits[:1], on_update=[])
            for w in waits[1:]:
                d2 = self.nc.sync.drain()
                d2.ins.sync_info = bass_rust.SyncInfo(on_wait=[w], on_update=[])
        self.nc.all_engine_barrier()
        popped = self.nc._tile_sem_poison_stack.pop()
        assert popped is self._sem_poison
        self.nc.all_engine_barrier()

    tile.TileContext._drain_and_barrier = _compat_drain_and_barrier
    tile.TileContext._bary_compat = True


_install_walrus_compat()

F32 = mybir.dt.float32

N_NODES = 64
M_QUERY = 2097152
N_CORES = 8
QC = M_QUERY // N_CORES      # 262144 queries per core
NG = 8                       # query groups packed into K=128
NP = N_NODES // 2            # 32 pairs (16 per half)
GQ8 = QC // NG               # 32768 queries per group per core
FCHUNK = 2048                # free-dim chunk width

# ---------------------------------------------------------------------------
# Custom DVE ops: q2-product fused with approx-reciprocal seed / NR steps.
# Registered at runtime into concourse.dve_ops.
# ---------------------------------------------------------------------------

_OPS = {}


def _register_custom_ops():
    if _OPS:
        return _OPS
    import concourse.dve_ops as dve_ops
    from concourse.dve_spec import Spec, Src0, Src1, C0, C1, C2, One, Bin, AluOp
    from concourse.dve_table_gen import dve_ver_for

    ver = dve_ver_for("TRN2")

    def _np_f32(v):
        return np.asarray(v, np.float32)

    # out = y1 where q=(x+c0)(x+c1); y0 = bitcast(~bits(q))*c2 ; y1 = y0*(2-q*y0)
    def _ref_seed(in0, in1, c0, c1, c2):
        x = _np_f32(in0)
        q = _np_f32(_np_f32(x + c0) * _np_f32(x + c1))
        nq = (~q.view(np.int32)).view(np.float32)
        y0 = _np_f32(nq * np.float32(c2))
        return _np_f32(y0 * _np_f32(np.float32(2.0) - _np_f32(q * y0)))

    # out = y*(2-q*y) with y = in1, q = (x+c0)(x+c1)
    def _ref_nr(in0, in1, c0, c1, c2):
        x = _np_f32(in0)
        y = _np_f32(in1)
        q = _np_f32(_np_f32(x + c0) * _np_f32(x + c1))
        return _np_f32(y * _np_f32(np.float32(2.0) - _np_f32(q * y)))

    a0 = Src0 + C0
    a1 = Src0 + C1
    q = a0 * a1
    nq = Bin(AluOp.BITWISE_NOT, q, q)
    y0 = nq * C2
    two = One + One  # stream-invariant, hoisted
    y1 = y0 * (two - q * y0)
    seed_spec = Spec(body=y1, reference=_ref_seed)

    qn = (Src0 + C0) * (Src0 + C1)
    nr_spec = Spec(body=Src1 * ((One + One) - qn * Src1), reference=_ref_nr)

    def _make(name, spec):
        if name not in dve_ops._SUB_OPCODE_FOR_NAME:
            dve_ops._SUB_OPCODE_FOR_NAME[name] = (
                dve_ops._CUSTOM_DVE_ROW_BASE + len(dve_ops.OPS)
            )
            dve_ops.OPS.append(None)  # placeholder keeps row indices stable
        row = dve_ops._SUB_OPCODE_FOR_NAME[name] - dve_ops._CUSTOM_DVE_ROW_BASE
        op = dve_ops.DveOp(name, spec, subdim=False, uops_sha={})
        try:
            op.compile(ver)
        except ValueError as e:
            m = re.search(r"drifted \((\w+): ([0-9a-f]+)", str(e))
            if not m:
                raise
            op = dve_ops.DveOp(name, spec, subdim=False,
                               uops_sha={m.group(1): m.group(2)})
        op.compile(ver)
        dve_ops.OPS[row] = op
        dve_ops.CUSTOM_DVE_SPECS[name] = spec
        return op

    # seed from u=(x-m)^2: q = u + c0(-d2); flip-seed + one Chebyshev NR
    def _ref_seed_u(in0, in1, c0, c1, c2):
        u = _np_f32(in0)
        q = _np_f32(u + c0)
        nq = (~q.view(np.int32)).view(np.float32)
        y0 = _np_f32(nq * np.float32(c2))
        return _np_f32(y0 * _np_f32(np.float32(c1) - _np_f32(q * y0)))

    qu = Src0 + C0
    nqu = Bin(AluOp.BITWISE_NOT, qu, qu)
    y0u = nqu * C2
    seedu_spec = Spec(body=y0u * (C1 - qu * y0u), reference=_ref_seed_u)

    seed_op = _make("BARY_PAIRQ_SEED", seed_spec)
    nr_op = _make("BARY_PAIRQ_NR", nr_spec)
    seedu_op = _make("BARY_SEED_U", seedu_spec)
    _OPS["seedu"] = seedu_op
    assert max(dve_ops._SUB_OPCODE_FOR_NAME.values()) < 0x20

    _OPS["seed"] = seed_op
    _OPS["nr"] = nr_op
    return _OPS


# ---------------------------------------------------------------------------
# Host-side coefficient preparation (float64, then rounded to f32)
# ---------------------------------------------------------------------------


def _host_coeffs(xi, fi, wi):
    xi = np.asarray(xi, np.float32)
    fi = np.asarray(fi, np.float32)
    wi = np.asarray(wi, np.float32)
    fw = (fi * wi).astype(np.float32)  # matches reference's f32 fi*wi

    # pair node P with node P+32: separation >= ~1.0 for Chebyshev nodes,
    # which keeps the (alpha*x + beta) linear form well-conditioned near roots
    a = xi[:32].astype(np.float64)
    b = xi[32:].astype(np.float64)
    fwa = fw[:32].astype(np.float64)
    fwb = fw[32:].astype(np.float64)
    wa = wi[:32].astype(np.float64)
    wb = wi[32:].astype(np.float64)

    alpha_f = (fwa + fwb).astype(np.float32)
    beta_f = (-(fwa * b + fwb * a)).astype(np.float32)
    alpha_w = (wa + wb).astype(np.float32)
    beta_w = (-(wa * b + wb * a)).astype(np.float32)

    # per-partition root constants; partition r = 8*p + g (pair-in-half p,
    # group g); column/stationary-half h selects pair P = 16*h + p.
    negA = np.zeros((128, 2), np.float32)
    negB = np.zeros((128, 2), np.float32)
    negM = np.zeros((128, 2), np.float32)   # -(a+b)/2
    negD2 = np.zeros((128, 2), np.float32)  # -((a-b)/2)^2
    S = np.zeros((128, 64), np.float32)
    for h in range(2):
        for p in range(16):
            P = 16 * h + p
            for g in range(8):
                r = 8 * p + g
                negA[r, h] = -xi[P]
                negB[r, h] = -xi[P + 32]
                negM[r, h] = np.float32(-(a[P] + b[P]) / 2.0)
                negD2[r, h] = np.float32(-((a[P] - b[P]) / 2.0) ** 2)
                S[r, 32 * h + 4 * g + 0] = alpha_f[P]
                S[r, 32 * h + 4 * g + 1] = beta_f[P]
                S[r, 32 * h + 4 * g + 2] = alpha_w[P]
                S[r, 32 * h + 4 * g + 3] = beta_w[P]
    return negA, negB, negM, negD2, S


# ---------------------------------------------------------------------------
# Bass module builder (parametric in group-queries / chunk size for testing)
# ---------------------------------------------------------------------------


def _region_epilogue(nc, epool, ubr, x_in, out_d, r, gq, fchunk):
    """Combine staged A,B,C,D for region r (4 chunks = 65536 queries):
    out = (x*A + B) / (x*C + D). Partition p = 32*cm + 8*k + g."""
    ES = 512
    xe = epool.tile([128, ES], F32, tag="xe")
    x_ap = bass.AP(
        tensor=x_in[:, :].tensor, offset=8192 * r,
        ap=[[2048, 4], [512, 4], [gq, 8], [1, ES]],
    )
    nc.sync.dma_start(xe[:, :], x_ap)

    A = ubr[:, 0 * ES:1 * ES]
    B = ubr[:, 1 * ES:2 * ES]
    C = ubr[:, 2 * ES:3 * ES]
    D = ubr[:, 3 * ES:4 * ES]

    t1 = epool.tile([128, ES], F32, tag="t1")
    nc.gpsimd.tensor_mul(t1[:, :], xe[:, :], A)
    numer = epool.tile([128, ES], F32, tag="numer")
    nc.gpsimd.tensor_add(numer[:, :], t1[:, :], B)
    t2 = epool.tile([128, ES], F32, tag="t2")
    nc.gpsimd.tensor_mul(t2[:, :], xe[:, :], C)
    denom = epool.tile([128, ES], F32, tag="denom")
    nc.gpsimd.tensor_add(denom[:, :], t2[:, :], D)

    scratch = epool.tile([128, ES], F32, tag="scr")
    rd = epool.tile([128, ES], F32, tag="rd")
    nc.vector.reciprocal_approx_accurate(rd[:, :], denom[:, :], scratch[:, :])
    o = epool.tile([128, ES], F32, tag="o")
    nc.gpsimd.tensor_mul(o[:, :], numer[:, :], rd[:, :])

    out_ap = bass.AP(
        tensor=out_d[:, :].tensor, offset=8192 * r,
        ap=[[2048, 4], [512, 4], [gq, 8], [1, ES]],
    )
    nc.sync.dma_start(out_ap, o[:, :])


def build_nc(gq=GQ8, fchunk=FCHUNK, mode="actq2", walrus_compat=True):
    """gq = queries per group (8 groups per core); fchunk = chunk width.
    mode: "actq2" (ScalarE computes q2, DVE does fast+NR recip) or
          "dve3" (3 fused custom DVE ops per half)."""
    ops = _register_custom_ops()
    assert gq % fchunk == 0
    nchunks = gq // fchunk
    nmm = fchunk // 512
    assert fchunk % 512 == 0
    assert nchunks * 8 <= 128

    nc = bass.Bass()
    x_in = nc.dram_tensor("x", [8, gq], F32, kind="ExternalInput")
    negA_in = nc.dram_tensor("negA", [128, 2], F32, kind="ExternalInput")
    negB_in = nc.dram_tensor("negB", [128, 2], F32, kind="ExternalInput")
    negM_in = nc.dram_tensor("negM", [128, 2], F32, kind="ExternalInput")
    negD2_in = nc.dram_tensor("negD2", [128, 2], F32, kind="ExternalInput")
    S_in = nc.dram_tensor("S", [128, 64], F32, kind="ExternalInput")
    out_d = nc.dram_tensor("out", [8, gq], F32, kind="ExternalOutput")

    npart = nchunks * 8

    with tile.TileContext(nc) as tc:
        tc._bary_walrus_compat = walrus_compat
        with (
            tc.tile_pool(name="const", bufs=1) as cpool,
            tc.tile_pool(name="xrep", bufs=4) as xpool,
            tc.tile_pool(name="ys", bufs=2) as ypool,
            tc.tile_pool(name="psum", bufs=2, space="PSUM") as ppool,
            tc.tile_pool(name="pstage", bufs=3) as pspool,
            tc.tile_pool(name="stage", bufs=1) as spool,
            tc.tile_pool(name="epi", bufs=2) as epool,
        ):
            negA_t = cpool.tile([128, 2], F32)
            nc.sync.dma_start(negA_t[:, :], negA_in[:, :])
            negB_t = cpool.tile([128, 2], F32)
            nc.sync.dma_start(negB_t[:, :], negB_in[:, :])
            negM_t = cpool.tile([128, 2], F32)
            nc.sync.dma_start(negM_t[:, :], negM_in[:, :])
            negD2_t = cpool.tile([128, 2], F32)
            nc.sync.dma_start(negD2_t[:, :], negD2_in[:, :])
            S_t = cpool.tile([128, 64], F32)
            nc.sync.dma_start(S_t[:, :], S_in[:, :])

            # uber staging: row 8c+g; quantity t at cols [t*fchunk,(t+1)*fchunk)
            uber = spool.tile([npart, 4 * fchunk], F32)
            ub = uber[:, :]
            pstride = ub.ap[0][0]

            for c in range(nchunks):
                xr = xpool.tile([128, fchunk], F32, tag="xr")
                src = x_in[:, c * fchunk:(c + 1) * fchunk].partition_broadcast(16)
                nc.sync.dma_start(xr[:, :], src)

                psum_t = ppool.tile([32, fchunk], F32, tag="ps")
                for h in range(2):
                    if mode == "actq2s":
                        u = ypool.tile([128, fchunk], F32, tag="u", bufs=3)
                        nc.scalar.activation(
                            u[:, :], xr[:, :],
                            mybir.ActivationFunctionType.Square,
                            bias=negM_t[:, h:h + 1],
                        )
                        y1 = ypool.tile([128, fchunk], F32, tag="y1")
                        nc.vector._custom_dve(
                            ops["seedu"], out=y1[:, :], in0=u[:, :],
                            s0=negD2_t[:, h:h + 1], s1=2.0017324,
                            imm2=-0.23549792,
                        )
                        y3 = ypool.tile([128, fchunk], F32, tag="y3", bufs=3)
                        nc.vector._custom_dve(
                            ops["nr"], out=y3[:, :], in0=xr[:, :], in1=y1[:, :],
                            s0=negA_t[:, h:h + 1], s1=negB_t[:, h:h + 1],
                        )
                    elif mode == "actq2":
                        u = ypool.tile([128, fchunk], F32, tag="u", bufs=3)
                        nc.scalar.activation(
                            u[:, :], xr[:, :],
                            mybir.ActivationFunctionType.Square,
                            bias=negM_t[:, h:h + 1],
                        )
                        q2 = ypool.tile([128, fchunk], F32, tag="q2")
                        nc.scalar.activation(
                            q2[:, :], u[:, :],
                            mybir.ActivationFunctionType.Identity,
                            bias=negD2_t[:, h:h + 1],
                        )
                        y1 = ypool.tile([128, fchunk], F32, tag="y1")
                        nc.vector.reciprocal_approx_fast(y1[:, :], q2[:, :])
                        # NR step against the exactly-computed q=(x-a)(x-b):
                        # fixes the (x-m)^2-d2 cancellation near roots
                        y3 = ypool.tile([128, fchunk], F32, tag="y3", bufs=3)
                        nc.vector._custom_dve(
                            ops["nr"], out=y3[:, :], in0=xr[:, :], in1=y1[:, :],
                            s0=negA_t[:, h:h + 1], s1=negB_t[:, h:h + 1],
                        )
                    else:
                        y1 = ypool.tile([128, fchunk], F32, tag="y")
                        nc.vector._custom_dve(
                            ops["seed"], out=y1[:, :], in0=xr[:, :],
                            s0=negA_t[:, h:h + 1], s1=negB_t[:, h:h + 1],
                            imm2=-4.0 / 17.0,
                        )
                        y2 = ypool.tile([128, fchunk], F32, tag="y")
                        nc.vector._custom_dve(
                            ops["nr"], out=y2[:, :], in0=xr[:, :], in1=y1[:, :],
                            s0=negA_t[:, h:h + 1], s1=negB_t[:, h:h + 1],
                        )
                        y3 = ypool.tile([128, fchunk], F32, tag="y")
                        nc.vector._custom_dve(
                            ops["nr"], out=y3[:, :], in0=xr[:, :], in1=y2[:, :],
                            s0=negA_t[:, h:h + 1], s1=negB_t[:, h:h + 1],
                        )
                    for k in range(nmm):
                        nc.tensor.matmul(
                            psum_t[:, 512 * k:512 * (k + 1)],
                            S_t[:, 32 * h:32 * (h + 1)],
                            y3[:, 512 * k:512 * (k + 1)],
                            start=(h == 0), stop=(h == 1),
                            skip_group_check=True,
                        )

                # ACT: psum -> sbuf (partition-preserving)
                ps_sb = pspool.tile([32, fchunk], F32, tag="pssb")
                nc.scalar.copy(ps_sb[:, :], psum_t[:, :])

                # DMA re-partition: ps_sb[4g+t, 512k+cc] -> uber[8c+g, t*fchunk+512k+cc]
                dst_ap = bass.AP(
                    tensor=ub.tensor,
                    offset=ub.offset + (8 * c) * pstride,
                    ap=[[pstride, 8], [fchunk, 4], [1, fchunk]],
                )
                eng = nc.sync if c % 2 == 0 else nc.gpsimd
                eng.dma_start(dst_ap, ps_sb[:, :])

            # tail epilogue over the whole core's queries, in 512-wide slices
            ES = 512
            nslice = fchunk // ES
            for sidx in range(nslice):
                xe = epool.tile([npart, ES], F32, tag="xe")
                x_ap = bass.AP(
                    tensor=x_in[:, :].tensor, offset=sidx * ES,
                    ap=[[fchunk, nchunks], [gq, 8], [1, ES]],
                )
                nc.sync.dma_start(xe[:, :], x_ap)

                sl = slice(sidx * ES, (sidx + 1) * ES)
                A = uber[:, 0 * fchunk:1 * fchunk][:, sl]
                B = uber[:, 1 * fchunk:2 * fchunk][:, sl]
                C = uber[:, 2 * fchunk:3 * fchunk][:, sl]
                D = uber[:, 3 * fchunk:4 * fchunk][:, sl]

                t1 = epool.tile([npart, ES], F32, tag="t1")
                nc.gpsimd.tensor_mul(t1[:, :], xe[:, :], A)
                numer = epool.tile([npart, ES], F32, tag="numer")
                nc.gpsimd.tensor_add(numer[:, :], t1[:, :], B)
                t2 = epool.tile([npart, ES], F32, tag="t2")
                nc.gpsimd.tensor_mul(t2[:, :], xe[:, :], C)
                denom = epool.tile([npart, ES], F32, tag="denom")
                nc.gpsimd.tensor_add(denom[:, :], t2[:, :], D)

                scratch = epool.tile([npart, ES], F32, tag="scr")
                rd = epool.tile([npart, ES], F32, tag="rd")
                nc.vector.reciprocal_approx_accurate(rd[:, :], denom[:, :], scratch[:, :])
                o = epool.tile([npart, ES], F32, tag="o")
                nc.gpsimd.tensor_mul(o[:, :], numer[:, :], rd[:, :])

                out_ap = bass.AP(
                    tensor=out_d[:, :].tensor, offset=sidx * ES,
                    ap=[[fchunk, nchunks], [gq, 8], [1, ES]],
                )
                nc.sync.dma_start(out_ap, o[:, :])

    if walrus_compat:
        _split_multiwait(nc)
    # populate .instr bytes for InstISA subclasses (custom DVE ops) — raw
    # Bass skips this pass and this walrus then rejects the empty encoding
    mybir.codegen_inst_isa_subclasses(nc)
    return nc


def _split_multiwait(nc):
    """This walrus accepts at most one sync-wait per instruction; hoist
    extras onto same-engine NOPs inserted just before the instruction."""
    import bass_rust
    from concourse.engine_type import EngineType

    ctr = [0]
    for fn in nc.m.functions:
        for bb in fn.blocks:
            il = bb.instructions
            out = []
            changed = False
            for inst in il:
                si = inst.sync_info
                waits = list(si.on_wait) if si is not None else []
                if len(waits) > 1:
                    changed = True
                    for w in waits[:-1]:
                        ctr[0] += 1
                        nop = mybir.InstNoOp(
                            name=f"mwsplit-{ctr[0]}", engine=inst.engine,
                            ins=[], outs=[],
                        )
                        nop.sync_info = bass_rust.SyncInfo(
                            on_wait=[w], on_update=[])
                        out.append(nop)
                    inst.sync_info = bass_rust.SyncInfo(
                        on_wait=[waits[-1]], on_update=list(si.on_update))
                out.append(inst)
            if changed:
                bb.instructions = out


# ---------------------------------------------------------------------------
# Host entry point
# ---------------------------------------------------------------------------

_NC_CACHE = {}


def _get_nc():
    if "nc" not in _NC_CACHE:
        _NC_CACHE["nc"] = build_nc()
    return _NC_CACHE["nc"]


def _fixup_nonfinite(out, x, xi, fi, wi):
    """Recompute non-finite outputs (exact hits -> NaN; denom==0 -> NaN)
    with a faithful f32 reference formula on the host."""
    bad = ~np.isfinite(out)
    # reference legitimately produces +-inf where its f32 denom rounds to 0;
    # our kernel produces NaN there (approx-recip of 0). Recompute every
    # non-finite lane with the straight f32 formula.
    if not bad.any():
        return out
    idx = np.nonzero(bad)[0]
    xb = x[idx].astype(np.float32)
    c = (xb[:, None] - xi[None, :]).astype(np.float32)
    z = c == 0.0
    cs = np.where(z, np.float32(1.0), c)
    r = (np.float32(1.0) / cs).astype(np.float32)
    fw = (fi * wi).astype(np.float32)
    numer = (r * fw[None, :]).sum(axis=1, dtype=np.float32)
    denom = (r * wi[None, :]).sum(axis=1, dtype=np.float32)
    vals = (numer / denom).astype(np.float32)
    hit = z.any(axis=1)
    if hit.any():
        vals[hit] = fi[z.argmax(axis=1)[hit]]
    out = out.copy()
    out[idx] = vals
    return out


def kernel(x, xi, fi, wi, _trace=False):
    x = np.asarray(x, np.float32)
    xi = np.asarray(xi, np.float32)
    fi = np.asarray(fi, np.float32)
    wi = np.asarray(wi, np.float32)

    negA, negB, negM, negD2, S = _host_coeffs(xi, fi, wi)
    nc = _get_nc()

    in_maps = []
    for cidx in range(N_CORES):
        xc = x[cidx * QC:(cidx + 1) * QC].reshape(NG, GQ8)
        in_maps.append({"x": xc, "negA": negA, "negB": negB,
                        "negM": negM, "negD2": negD2, "S": S})

    res = run_bass_kernel_spmd(
        nc, in_maps, core_ids=list(range(N_CORES)), trace=_trace,
    )
    out = np.concatenate([r["out"].reshape(-1) for r in res.results])
    out = _fixup_nonfinite(out, x, xi, fi, wi)
    if _trace:
        kernel._last_exec_time_ns = res.exec_time_ns
        kernel._last_results = res
    return out
